# revision 3
# baseline (speedup 1.0000x reference)
"""Bass/Trainium2 kernel for nn_AttentionModule (additive attention scorer).

reference math (B=64, S=2048, D=512):
    q = query @ Wq_w.T + Wq_b                       # [B, D]
    r = einsum("bsd,ed->bse", ref, Wr_w) + Wr_b     # [B, S, D]
    h = tanh(q[:, None, :] + r)
    logits = 10 * tanh(einsum("bsd,d->bs", h, value))
    returns (r.transpose(0, 2, 1), logits)          # [B, D, S], [B, S]

Strategy: pure data-parallel over batch across the 8 NeuronCores (8
batches/core, no collectives).  Per core everything is computed in the
TRANSPOSED layout rT[dout, s], which makes every post-matmul op natural:
 - ref tiles [128 s, D] are PE-transposed to refT [128 din, s]
 - rT[dout, s] = WrT.T @ refT accumulated over 4 din chunks (f32r matmuls,
   1 cyc/row at N=512 vs 4 for f32; ~1.6e-4 rel err)
 - Wr_b / (q + Wq_b + Wr_b) are per-partition bias vectors: fused into a
   DVE tensor_scalar add (r out) and a ScalarE tanh activation (h)
 - logits = value . h is a partition-dim reduction: matmul with the value
   chunk as a [128, 1] stationary operand, accumulated over dout chunks
"""

import sys

sys.path.insert(0, "/opt/trn_rl_repo")

import numpy as np  # noqa: E402
import concourse.bass as bass  # noqa: E402
import concourse.mybir as mybir  # noqa: E402
import concourse.tile as tile  # noqa: E402
from concourse.bass_utils import run_bass_kernel_spmd  # noqa: E402
from concourse.masks import make_identity  # noqa: E402

N_CORES = 8
B, S, D = 64, 2048, 512
BS = B // N_CORES  # batches per core
SB = 512  # s-block (free dim of the main matmuls)
NSB = S // SB  # s-blocks per batch
NC4 = D // 128  # 128-sized chunks of D
C_LOGIT = 10.0

F32 = mybir.dt.float32
F32R = mybir.dt.float32r


def _split_multiwaits(nc):
    """walrus in this container caps sync waits at 1/instruction; split
    multi-wait instructions into single-wait NOPs + the instruction."""
    k = 0
    for fn in nc.m.functions:
        for bb in fn.blocks:
            insts = bb.instructions
            if not any(i.sync_info and len(i.sync_info.on_wait) > 1 for i in insts):
                continue
            out = []
            for inst in insts:
                si = inst.sync_info
                if si is not None and len(si.on_wait) > 1:
                    waits = list(si.on_wait)
                    for w in waits[:-1]:
                        out.append(
                            mybir.InstNoOp(
                                name=f"waitsplit_{k}",
                                engine=inst.engine,
                                bass_nofuse=True,
                                sync_info=mybir.SyncInfo(on_wait=[w], on_update=[]),
                            )
                        )
                        k += 1
                    inst.sync_info = mybir.SyncInfo(
                        on_wait=[waits[-1]], on_update=list(si.on_update)
                    )
                out.append(inst)
            bb.instructions = out
    return k


def _transpose_512(nc, pool_ps, pool_out, w_nat, ident, name, dtype=F32R):
    """w_nat: [128, 4, 512] natural ([row_p, row_chunk, col]) ->
    returns tile [128, 4, 512] = [col_p, col_chunk, row] (rounded copy)."""
    wt = pool_out.tile([128, NC4, D], dtype, name=name)
    for i in range(NC4):
        wt_ps = pool_ps.tile([128, D], F32, tag="wt_ps", bufs=2)
        for c in range(NC4):
            nc.tensor.transpose(
                wt_ps[:, c * 128 : (c + 1) * 128],
                w_nat[:, c, i * 128 : (i + 1) * 128],
                ident[:],
            )
        nc.vector.tensor_copy(wt[:, i, :], wt_ps[:])
    return wt


def build_nc():
    nc = bass.Bass()
    query = nc.declare_dram_parameter("query", [BS, D], F32, isOutput=False)
    ref = nc.declare_dram_parameter("ref", [BS, S, D], F32, isOutput=False)
    Wq_w = nc.declare_dram_parameter("Wq_w", [D, D], F32, isOutput=False)
    Wq_b = nc.declare_dram_parameter("Wq_b", [D], F32, isOutput=False)
    Wr_w = nc.declare_dram_parameter("Wr_w", [D, D], F32, isOutput=False)
    Wr_b = nc.declare_dram_parameter("Wr_b", [D], F32, isOutput=False)
    value = nc.declare_dram_parameter("value", [D], F32, isOutput=False)
    ref_out = nc.declare_dram_parameter("ref_out", [BS, D, S], F32, isOutput=True)
    logits = nc.declare_dram_parameter("logits", [BS, S], F32, isOutput=True)

    with tile.TileContext(nc) as tc:
        with tc.tile_pool(name="consts", bufs=1) as consts:
            ident = consts.tile([128, 128], F32)
            make_identity(nc, ident)

            # biases / value in [128 part, chunk] layout
            wrb_sb = consts.tile([128, NC4], F32)
            nc.sync.dma_start(out=wrb_sb[:], in_=Wr_b.rearrange("(c p) -> p c", p=128))
            wqb_sb = consts.tile([128, NC4], F32)
            nc.sync.dma_start(out=wqb_sb[:], in_=Wq_b.rearrange("(c p) -> p c", p=128))
            val_r = consts.tile([128, NC4], F32R)
            val_f = consts.tile([128, NC4], F32)
            nc.sync.dma_start(out=val_f[:], in_=value.rearrange("(c p) -> p c", p=128))
            nc.vector.tensor_copy(val_r[:], val_f[:])
            # wb = Wq_b + Wr_b (bias of h's tanh, per dout partition)
            wb_sb = consts.tile([128, NC4], F32)
            nc.vector.tensor_add(wb_sb[:], wqb_sb[:], wrb_sb[:])

            # qb_sb[j][:, b] = (Wq q_b + Wq_b + Wr_b) chunk j  -> [128, 4, BS]
            qb_sb = consts.tile([128, NC4, BS], F32)

            with (
                tc.tile_pool(name="setup_sb", bufs=1) as ssb,
                tc.tile_pool(name="setup_ps", bufs=2, space="PSUM") as sps,
            ):
                wr_nat = ssb.tile([128, NC4, D], F32)
                nc.sync.dma_start(
                    out=wr_nat[:], in_=Wr_w.rearrange("(c p) d -> p c d", p=128)
                )
                WrT = _transpose_512(nc, sps, consts, wr_nat, ident, "WrT")
                wq_nat = ssb.tile([128, NC4, D], F32)
                nc.sync.dma_start(
                    out=wq_nat[:], in_=Wq_w.rearrange("(c p) d -> p c d", p=128)
                )
                WqT = _transpose_512(nc, sps, ssb, wq_nat, ident, "WqT")

                # q = query @ Wq_w.T : [128 dout, BS] chunks
                q_nat = ssb.tile([BS, D], F32)
                nc.sync.dma_start(out=q_nat[:], in_=query[:])
                qT_sbs = []
                for i in range(NC4):
                    qT_ps = sps.tile([128, BS], F32, tag="qT_ps", bufs=2)
                    nc.tensor.transpose(
                        qT_ps[:], q_nat[:, i * 128 : (i + 1) * 128], ident[:BS, :BS]
                    )
                    qT_sb = ssb.tile([128, BS], F32R, tag="qT_sb", bufs=NC4)
                    nc.vector.tensor_copy(qT_sb[:], qT_ps[:])
                    qT_sbs.append(qT_sb)
                for j in range(NC4):
                    q_ps = sps.tile([128, BS], F32, tag="q_ps", bufs=2)
                    for i in range(NC4):
                        nc.tensor.matmul(
                            q_ps[:],
                            WqT[:, i, j * 128 : (j + 1) * 128],
                            qT_sbs[i][:],
                            start=(i == 0),
                            stop=(i == NC4 - 1),
                        )
                    nc.vector.tensor_scalar_add(
                        qb_sb[:, j, :], q_ps[:], wb_sb[:, j : j + 1]
                    )

            # ---- main loop ----
            with (
                tc.tile_pool(name="io", bufs=3) as io,
                tc.tile_pool(name="mid", bufs=1) as mid,
                tc.tile_pool(name="psA", bufs=2, space="PSUM") as psA,
                tc.tile_pool(name="psB", bufs=2, space="PSUM") as psB,
                tc.tile_pool(name="psC", bufs=2, space="PSUM") as psC,
            ):
                for b in range(BS):
                    lg_sb = mid.tile([1, S], F32, tag="lg_sb", bufs=2)
                    for sb in range(NSB):
                        ref_nat = io.tile([128, NSB, D], F32, tag="ref_nat", bufs=3)
                        nc.sync.dma_start(
                            out=ref_nat[:],
                            in_=ref[b, sb * SB : (sb + 1) * SB, :].rearrange(
                                "(t p) d -> p t d", p=128
                            ),
                        )
                        refT_sbs = []
                        for i in range(NC4):
                            refT_ps = psA.tile([128, SB], F32, tag="refT_ps", bufs=2)
                            for t in range(SB // 128):
                                nc.tensor.transpose(
                                    refT_ps[:, t * 128 : (t + 1) * 128],
                                    ref_nat[:, t, i * 128 : (i + 1) * 128],
                                    ident[:],
                                )
                            refT_sb = mid.tile(
                                [128, SB], F32R, tag="refT_sb", bufs=2 * NC4
                            )
                            nc.vector.tensor_copy(refT_sb[:], refT_ps[:])
                            refT_sbs.append(refT_sb)

                        lg_ps = psC.tile([1, SB], F32, tag="lg_ps", bufs=2)
                        for j in range(NC4):
                            rT_ps = psB.tile([128, SB], F32, tag="rT_ps", bufs=2)
                            for i in range(NC4):
                                nc.tensor.matmul(
                                    rT_ps[:],
                                    WrT[:, i, j * 128 : (j + 1) * 128],
                                    refT_sbs[i][:],
                                    start=(i == 0),
                                    stop=(i == NC4 - 1),
                                )
                            # r = rT + Wr_b  -> DRAM (ref_out is r transposed)
                            r_sb = io.tile([128, SB], F32, tag="r_sb", bufs=3)
                            nc.vector.tensor_scalar_add(
                                r_sb[:], rT_ps[:], wrb_sb[:, j : j + 1]
                            )
                            nc.gpsimd.dma_start(
                                out=ref_out[
                                    b, j * 128 : (j + 1) * 128, sb * SB : (sb + 1) * SB
                                ],
                                in_=r_sb[:],
                            )
                            # h = tanh(rT + (q + Wq_b + Wr_b))
                            h_sb = mid.tile([128, SB], F32R, tag="h_sb", bufs=2)
                            nc.scalar.activation(
                                h_sb[:],
                                rT_ps[:],
                                mybir.ActivationFunctionType.Tanh,
                                bias=qb_sb[:, j, b : b + 1],
                            )
                            # logits partial: value_j . h_j
                            nc.tensor.matmul(
                                lg_ps[:],
                                val_r[:, j : j + 1],
                                h_sb[:],
                                start=(j == 0),
                                stop=(j == NC4 - 1),
                            )
                        nc.vector.tensor_copy(
                            lg_sb[:, sb * SB : (sb + 1) * SB], lg_ps[:]
                        )
                    # logits_b = 10 * tanh(lg)
                    lgt_sb = mid.tile([1, S], F32, tag="lgt_sb", bufs=2)
                    nc.scalar.activation(
                        lgt_sb[:], lg_sb[:], mybir.ActivationFunctionType.Tanh
                    )
                    nc.vector.tensor_scalar_mul(lgt_sb[:], lgt_sb[:], C_LOGIT)
                    nc.gpsimd.dma_start(out=logits[b : b + 1, :], in_=lgt_sb[:])

    _split_multiwaits(nc)
    return nc


_NC = None
TRACE = False  # set True (e.g. from test.py) to neuron-profile the run
LAST = None  # BassKernelResults of the last kernel() call when TRACE


def kernel(query, ref, Wq_w, Wq_b, Wr_w, Wr_b, value):
    global _NC, LAST
    query = np.ascontiguousarray(query, dtype=np.float32)
    ref = np.ascontiguousarray(ref, dtype=np.float32)
    shared = {
        "Wq_w": np.ascontiguousarray(Wq_w, dtype=np.float32),
        "Wq_b": np.ascontiguousarray(Wq_b, dtype=np.float32),
        "Wr_w": np.ascontiguousarray(Wr_w, dtype=np.float32),
        "Wr_b": np.ascontiguousarray(Wr_b, dtype=np.float32),
        "value": np.ascontiguousarray(value, dtype=np.float32),
    }
    if _NC is None:
        _NC = build_nc()
    in_maps = [
        {
            "query": query[c * BS : (c + 1) * BS],
            "ref": ref[c * BS : (c + 1) * BS],
            **shared,
        }
        for c in range(N_CORES)
    ]
    res = run_bass_kernel_spmd(
        _NC, in_maps, core_ids=list(range(N_CORES)), trace=TRACE
    )
    LAST = res
    ref_out = np.concatenate([res.results[c]["ref_out"] for c in range(N_CORES)], 0)
    logits = np.concatenate([res.results[c]["logits"] for c in range(N_CORES)], 0)
    return ref_out, logits


# revision 7
# speedup vs baseline: 1.4368x; 1.4368x over previous
"""Bass/Trainium2 kernel for nn_AttentionModule (additive attention scorer).

reference math (B=64, S=2048, D=512):
    q = query @ Wq_w.T + Wq_b                       # [B, D]
    r = einsum("bsd,ed->bse", ref, Wr_w) + Wr_b     # [B, S, D]
    h = tanh(q[:, None, :] + r)
    logits = 10 * tanh(einsum("bsd,d->bs", h, value))
    returns (r.transpose(0, 2, 1), logits)          # [B, D, S], [B, S]

Strategy: pure data-parallel over batch across the 8 NeuronCores (8
batches/core, no collectives).  Per core everything is computed in the
TRANSPOSED layout rT[dout, s], which makes every post-matmul op natural:
 - ref tiles [128 s, D] are PE-transposed to refT [128 din, s]
 - rT[dout, s] = WrT.T @ refT accumulated over 4 din chunks (f32r matmuls,
   1 cyc/row at N=512 vs 4 for f32; ~1.6e-4 rel err)
 - Wr_b / (q + Wq_b + Wr_b) are per-partition bias vectors: fused into a
   DVE tensor_scalar add (r out) and a ScalarE tanh activation (h)
 - logits = value . h is a partition-dim reduction: matmul with the value
   chunk as a [128, 1] stationary operand, accumulated over dout chunks
"""

import sys

sys.path.insert(0, "/opt/trn_rl_repo")

import numpy as np  # noqa: E402
import concourse.bass as bass  # noqa: E402
import concourse.mybir as mybir  # noqa: E402
import concourse.tile as tile  # noqa: E402
from concourse.bass_utils import run_bass_kernel_spmd  # noqa: E402
from concourse.masks import make_identity  # noqa: E402

N_CORES = 8
B, S, D = 64, 2048, 512
BS = B // N_CORES  # batches per core
SB = 512  # s-block (free dim of the main matmuls)
NSB = S // SB  # s-blocks per batch
NC4 = D // 128  # 128-sized chunks of D
C_LOGIT = 10.0

F32 = mybir.dt.float32
F32R = mybir.dt.float32r


def _split_multiwaits(nc):
    """walrus in this container caps sync waits at 1/instruction; split
    multi-wait instructions into single-wait NOPs + the instruction."""
    k = 0
    for fn in nc.m.functions:
        for bb in fn.blocks:
            insts = bb.instructions
            if not any(i.sync_info and len(i.sync_info.on_wait) > 1 for i in insts):
                continue
            out = []
            for inst in insts:
                si = inst.sync_info
                if si is not None and len(si.on_wait) > 1:
                    waits = list(si.on_wait)
                    for w in waits[:-1]:
                        out.append(
                            mybir.InstNoOp(
                                name=f"waitsplit_{k}",
                                engine=inst.engine,
                                bass_nofuse=True,
                                sync_info=mybir.SyncInfo(on_wait=[w], on_update=[]),
                            )
                        )
                        k += 1
                    inst.sync_info = mybir.SyncInfo(
                        on_wait=[waits[-1]], on_update=list(si.on_update)
                    )
                out.append(inst)
            bb.instructions = out
    return k


def _transpose_512(nc, pool_ps, pool_out, w_nat, ident, name, dtype=F32R):
    """w_nat: [128, 4, 512] natural ([row_p, row_chunk, col]) ->
    returns tile [128, 4, 512] = [col_p, col_chunk, row] (rounded copy)."""
    wt = pool_out.tile([128, NC4, D], dtype, name=name)
    for i in range(NC4):
        wt_ps = pool_ps.tile([128, D], F32, tag="wt_ps", bufs=2)
        for c in range(NC4):
            nc.tensor.transpose(
                wt_ps[:, c * 128 : (c + 1) * 128],
                w_nat[:, c, i * 128 : (i + 1) * 128],
                ident[:],
            )
        nc.vector.tensor_copy(wt[:, i, :], wt_ps[:])
    return wt


def build_nc():
    nc = bass.Bass()
    query = nc.declare_dram_parameter("query", [BS, D], F32, isOutput=False)
    ref = nc.declare_dram_parameter("ref", [BS, S, D], F32, isOutput=False)
    Wq_w = nc.declare_dram_parameter("Wq_w", [D, D], F32, isOutput=False)
    Wq_b = nc.declare_dram_parameter("Wq_b", [D], F32, isOutput=False)
    Wr_w = nc.declare_dram_parameter("Wr_w", [D, D], F32, isOutput=False)
    Wr_b = nc.declare_dram_parameter("Wr_b", [D], F32, isOutput=False)
    value = nc.declare_dram_parameter("value", [D], F32, isOutput=False)
    ref_out = nc.declare_dram_parameter("ref_out", [BS, D, S], F32, isOutput=True)
    logits = nc.declare_dram_parameter("logits", [BS, S], F32, isOutput=True)

    with tile.TileContext(nc) as tc:
        with tc.tile_pool(name="consts", bufs=1) as consts:
            ident = consts.tile([128, 128], F32)
            make_identity(nc, ident)

            # biases / value in [128 part, chunk] layout
            wrb_sb = consts.tile([128, NC4], F32)
            nc.sync.dma_start(out=wrb_sb[:], in_=Wr_b.rearrange("(c p) -> p c", p=128))
            wqb_sb = consts.tile([128, NC4], F32)
            nc.sync.dma_start(out=wqb_sb[:], in_=Wq_b.rearrange("(c p) -> p c", p=128))
            val_r = consts.tile([128, NC4], F32R)
            val_f = consts.tile([128, NC4], F32)
            nc.sync.dma_start(out=val_f[:], in_=value.rearrange("(c p) -> p c", p=128))
            nc.vector.tensor_copy(val_r[:], val_f[:])
            # wb = Wq_b + Wr_b (bias of h's tanh, per dout partition)
            wb_sb = consts.tile([128, NC4], F32)
            nc.vector.tensor_add(wb_sb[:], wqb_sb[:], wrb_sb[:])

            # qb_sb[j][:, b] = (Wq q_b + Wq_b + Wr_b) chunk j  -> [128, 4, BS]
            qb_sb = consts.tile([128, NC4, BS], F32)

            with (
                tc.tile_pool(name="setup_sb", bufs=1) as ssb,
                tc.tile_pool(name="setup_ps", bufs=2, space="PSUM") as sps,
            ):
                wr_nat = ssb.tile([128, NC4, D], F32)
                nc.sync.dma_start(
                    out=wr_nat[:], in_=Wr_w.rearrange("(c p) d -> p c d", p=128)
                )
                WrT = _transpose_512(nc, sps, consts, wr_nat, ident, "WrT")
                wq_nat = ssb.tile([128, NC4, D], F32)
                nc.sync.dma_start(
                    out=wq_nat[:], in_=Wq_w.rearrange("(c p) d -> p c d", p=128)
                )
                WqT = _transpose_512(nc, sps, ssb, wq_nat, ident, "WqT")

                # q = query @ Wq_w.T : [128 dout, BS] chunks
                q_nat = ssb.tile([BS, D], F32)
                nc.sync.dma_start(out=q_nat[:], in_=query[:])
                qT_sbs = []
                for i in range(NC4):
                    qT_ps = sps.tile([128, BS], F32, tag="qT_ps", bufs=2)
                    nc.tensor.transpose(
                        qT_ps[:], q_nat[:, i * 128 : (i + 1) * 128], ident[:BS, :BS]
                    )
                    qT_sb = ssb.tile([128, BS], F32R, tag="qT_sb", bufs=NC4)
                    nc.vector.tensor_copy(qT_sb[:], qT_ps[:])
                    qT_sbs.append(qT_sb)
                for j in range(NC4):
                    q_ps = sps.tile([128, BS], F32, tag="q_ps", bufs=2)
                    for i in range(NC4):
                        nc.tensor.matmul(
                            q_ps[:],
                            WqT[:, i, j * 128 : (j + 1) * 128],
                            qT_sbs[i][:],
                            start=(i == 0),
                            stop=(i == NC4 - 1),
                        )
                    nc.vector.tensor_scalar_add(
                        qb_sb[:, j, :], q_ps[:], wb_sb[:, j : j + 1]
                    )

            # ---- main loop ----
            with (
                tc.tile_pool(name="io", bufs=4) as io,
                tc.tile_pool(name="mid", bufs=1) as mid,
                tc.tile_pool(name="psA", bufs=3, space="PSUM") as psA,
                tc.tile_pool(name="psB", bufs=3, space="PSUM") as psB,
                tc.tile_pool(name="psC", bufs=2, space="PSUM") as psC,
            ):
                for b in range(BS):
                    lg_sb = mid.tile([1, S], F32, tag="lg_sb", bufs=2)
                    for sb in range(NSB):
                        ref_nat = io.tile([128, NSB, D], F32, tag="ref_nat", bufs=4)
                        nc.sync.dma_start(
                            out=ref_nat[:],
                            in_=ref[b, sb * SB : (sb + 1) * SB, :].rearrange(
                                "(t p) d -> p t d", p=128
                            ),
                        )
                        refT_sbs = []
                        for i in range(NC4):
                            refT_ps = psA.tile([128, SB], F32, tag="refT_ps", bufs=3)
                            for t in range(SB // 128):
                                nc.tensor.transpose(
                                    refT_ps[:, t * 128 : (t + 1) * 128],
                                    ref_nat[:, t, i * 128 : (i + 1) * 128],
                                    ident[:],
                                )
                            refT_sb = mid.tile(
                                [128, SB], F32R, tag="refT_sb", bufs=3 * NC4
                            )
                            nc.vector.tensor_copy(refT_sb[:], refT_ps[:])
                            refT_sbs.append(refT_sb)

                        h_sbs = []
                        for j in range(NC4):
                            rT_ps = psB.tile([128, SB], F32, tag="rT_ps", bufs=3)
                            for i in range(NC4):
                                nc.tensor.matmul(
                                    rT_ps[:],
                                    WrT[:, i, j * 128 : (j + 1) * 128],
                                    refT_sbs[i][:],
                                    start=(i == 0),
                                    stop=(i == NC4 - 1),
                                )
                            # r = rT + Wr_b  -> DRAM (ref_out is r transposed)
                            r_sb = io.tile([128, SB], F32, tag="r_sb", bufs=4)
                            if j < 2:
                                nc.vector.tensor_scalar_add(
                                    r_sb[:], rT_ps[:], wrb_sb[:, j : j + 1]
                                )
                            else:
                                nc.scalar.activation(
                                    r_sb[:],
                                    rT_ps[:],
                                    mybir.ActivationFunctionType.Identity,
                                    bias=wrb_sb[:, j : j + 1],
                                )
                            nc.gpsimd.dma_start(
                                out=ref_out[
                                    b, j * 128 : (j + 1) * 128, sb * SB : (sb + 1) * SB
                                ],
                                in_=r_sb[:],
                            )
                            # h = tanh(rT + (q + Wq_b + Wr_b))
                            h_sb = mid.tile([128, SB], F32R, tag="h_sb", bufs=2 * NC4)
                            nc.scalar.activation(
                                h_sb[:],
                                rT_ps[:],
                                mybir.ActivationFunctionType.Tanh,
                                bias=qb_sb[:, j, b : b + 1],
                            )
                            h_sbs.append(h_sb)
                        # logits: value . h, accumulated over the 4 dout chunks
                        lg_ps = psC.tile([1, SB], F32, tag="lg_ps", bufs=2)
                        for j in range(NC4):
                            nc.tensor.matmul(
                                lg_ps[:],
                                val_r[:, j : j + 1],
                                h_sbs[j][:],
                                start=(j == 0),
                                stop=(j == NC4 - 1),
                            )
                        nc.vector.tensor_copy(
                            lg_sb[:, sb * SB : (sb + 1) * SB], lg_ps[:]
                        )
                    # logits_b = 10 * tanh(lg)
                    lgt_sb = mid.tile([1, S], F32, tag="lgt_sb", bufs=2)
                    nc.scalar.activation(
                        lgt_sb[:], lg_sb[:], mybir.ActivationFunctionType.Tanh
                    )
                    nc.vector.tensor_scalar_mul(lgt_sb[:], lgt_sb[:], C_LOGIT)
                    nc.gpsimd.dma_start(out=logits[b : b + 1, :], in_=lgt_sb[:])

    _split_multiwaits(nc)
    return nc


_NC = None
TRACE = False  # set True (e.g. from test.py) to neuron-profile the run
LAST = None  # BassKernelResults of the last kernel() call when TRACE


def kernel(query, ref, Wq_w, Wq_b, Wr_w, Wr_b, value):
    global _NC, LAST
    query = np.ascontiguousarray(query, dtype=np.float32)
    ref = np.ascontiguousarray(ref, dtype=np.float32)
    shared = {
        "Wq_w": np.ascontiguousarray(Wq_w, dtype=np.float32),
        "Wq_b": np.ascontiguousarray(Wq_b, dtype=np.float32),
        "Wr_w": np.ascontiguousarray(Wr_w, dtype=np.float32),
        "Wr_b": np.ascontiguousarray(Wr_b, dtype=np.float32),
        "value": np.ascontiguousarray(value, dtype=np.float32),
    }
    if _NC is None:
        _NC = build_nc()
    in_maps = [
        {
            "query": query[c * BS : (c + 1) * BS],
            "ref": ref[c * BS : (c + 1) * BS],
            **shared,
        }
        for c in range(N_CORES)
    ]
    res = run_bass_kernel_spmd(
        _NC, in_maps, core_ids=list(range(N_CORES)), trace=TRACE
    )
    LAST = res
    ref_out = np.concatenate([res.results[c]["ref_out"] for c in range(N_CORES)], 0)
    logits = np.concatenate([res.results[c]["logits"] for c in range(N_CORES)], 0)
    return ref_out, logits
